# revision 1
# baseline (speedup 1.0000x reference)
"""ANI-style MoE routing kernel for 8 Trainium2 NeuronCores.

Strategy (data-parallel + host routing):
  - Host: sort atoms by type, split each type's atom list evenly across the
    8 cores, and build per-core per-expert contiguous batches padded to
    adaptive per-expert capacities (multiples of 512).  Batches are stored
    pre-transposed (feature-major, [384, sum(caps)]) in bf16 so the device
    streams them straight into matmuls.
  - Device (per core): for each expert, a 3-layer MLP in feature-major
    layout with bf16 matmul operands (f32 PSUM accumulate), emitted as a
    2-stage software pipeline so the PE never queue-stalls on pointwise ops.
    CELU is exact:
      layer 1:  g = max(z, min(exp(z)-1, 0))     [ACT exp -> DVE ts -> DVE tt]
      layer 2:  h = relu(z) + min(exp(z)-1, 0)   [r on DVE; q's main half is
                computed on ACT as relu(1-exp(z)) = -q with the negation
                folded into a negated-W3 lhsT in layer 3; the r+q add is
                folded into layer 3's matmul as two rhs parts]
    exp outputs stay f32 (bf16 near 1.0 would wreck expm1 precision);
    pointwise ops run on packed [128, 1024] PSUM views (the m-tile pair),
    tolerating never-read garbage lanes.  Layer 3 accumulates all per-atom
    energies into one [1, 512] PSUM tile across all chunks; a single
    free-axis reduce yields the core's partial energy.
  - Host: sum the 8 partial scalars (+ exact f64 corrections for padding
    rows and the b3 bias, both computable host-side from the weights).
    Batches that barely overflow a 512 boundary (<= 64 atoms/core) shed the
    overflow to an exact f64 host computation instead of paying a whole
    extra device chunk (66 -> 64 chunks on the reference distribution).

Zero-padding rows pass through the MLP to exactly zero energy when
b1 == b2 == 0 (always true for this problem's init); the general-bias path
adds per-layer bias matmuls and corrects the pad contribution on the host.
"""

import os
import sys

import numpy as np

try:
    import concourse.bass as bass  # noqa: F401
except ImportError:  # pragma: no cover
    sys.path.insert(0, "/opt/trn_rl_repo")
    import concourse.bass as bass  # noqa: F401

import concourse.mybir as mybir
import concourse.tile as tile
from concourse import bacc
from concourse import bass_utils

IN_DIM = 384
H1 = 192
H2 = 160
E = 4
N_CORES = 8
N_ATOMS = 262144

CHUNK = 512

F32 = mybir.dt.float32
F32R = mybir.dt.float32r
BF16 = mybir.dt.bfloat16
AF = mybir.ActivationFunctionType
ALU = mybir.AluOpType

USE_BF16 = os.environ.get("BF16", "1") == "1"
MMDT = BF16 if USE_BF16 else F32R  # matmul operand dtype
IODT_NP = None  # set lazily in prepare_in_maps


def _np_mmdt():
    if USE_BF16:
        import ml_dtypes

        return ml_dtypes.bfloat16
    return np.float32


def _build_graph(with_bias: bool, caps, repeat: int = 1):
    nc = bacc.Bacc(
        "TRN2",
        target_bir_lowering=False,
        debug=False,
        enable_asserts=False,
        num_devices=N_CORES,
    )
    total_cap = sum(caps)
    iodt = BF16 if USE_BF16 else F32
    xT = nc.dram_tensor("xT", [IN_DIM, total_cap], iodt, kind="ExternalInput").ap()
    W1 = nc.dram_tensor("W1", [E, IN_DIM, H1], iodt, kind="ExternalInput").ap()
    W2 = nc.dram_tensor("W2", [E, H1, H2], iodt, kind="ExternalInput").ap()
    W3 = nc.dram_tensor("W3", [E, H2, 1], iodt, kind="ExternalInput").ap()
    B1 = B2 = None
    if with_bias:
        bdt = BF16 if USE_BF16 else F32
        B1 = nc.dram_tensor("B1", [E, H1], bdt, kind="ExternalInput").ap()
        B2 = nc.dram_tensor("B2", [E, H2], bdt, kind="ExternalInput").ap()
    out = nc.dram_tensor("out", [1, 1], F32, kind="ExternalOutput").ap()

    with tile.TileContext(nc) as tc:
        _emit(tc, xT, W1, W2, W3, B1, B2, out, with_bias, caps, repeat)
    nc.compile()
    return nc


def _emit(tc, xT, W1, W2, W3, B1, B2, out, with_bias, caps, repeat=1):
    nc = tc.nc
    xT3 = xT.rearrange("(kt kp) n -> kp kt n", kp=128)  # [128, 3, E*CAP]

    with (
        tc.tile_pool(name="wpool", bufs=2) as wp,
        tc.tile_pool(name="xpool", bufs=int(os.environ.get("X_BUFS", "4"))) as xp,
        tc.tile_pool(name="hpool", bufs=int(os.environ.get("H_BUFS", "6"))) as hp,
        tc.tile_pool(name="gpool", bufs=int(os.environ.get("G_BUFS", "3"))) as gp,
        tc.tile_pool(
            name="zpool", bufs=int(os.environ.get("Z1_BUFS", "2")), space="PSUM"
        ) as zp,
        tc.tile_pool(
            name="z2pool", bufs=int(os.environ.get("Z2_BUFS", "1")), space="PSUM"
        ) as z2p,
        tc.tile_pool(name="accpool", bufs=1, space="PSUM") as accp,
        tc.tile_pool(name="cpool", bufs=1) as cp,
    ):
        import contextlib

        loop_cm = tc.For_i(0, repeat, 1) if repeat > 1 else contextlib.nullcontext()
        with loop_cm:
            _emit_body(
                tc, xT3, W1, W2, W3, B1, B2, out, with_bias, caps,
                wp, xp, hp, gp, zp, z2p, accp, cp,
            )


def _emit_body(
    tc, xT3, W1, W2, W3, B1, B2, out, with_bias, caps,
    wp, xp, hp, gp, zp, z2p, accp, cp,
):
    nc = tc.nc
    acc = accp.tile([1, CHUNK], F32, tag="acc")  # persistent energy accumulator
    ones = None
    if with_bias:
        ones = cp.tile([1, CHUNK], MMDT, tag="ones")
        nc.vector.memset(ones[:], 1.0)

    celu_mode = os.environ.get("CELU_MODE", "full")
    dma_add = os.environ.get("DMAADD", "0") == "1"

    def celu_max(zfull, g):
        """g = celu(z) = max(z, min(exp(z)-1, 0)) on a packed [128, 2*CHUNK]
        view (garbage lanes tolerated; never read downstream)."""
        if celu_mode == "copy":  # timing-skeleton variant (wrong numerics)
            nc.scalar.activation(g[:], zfull, AF.Copy)
            return
        t = hp.tile([128, 2 * CHUNK], F32, tag="t")
        nc.scalar.activation(t[:], zfull, AF.Exp)
        if os.environ.get("U1ACT", "0") == "1":
            # un = relu(1 - exp(z)) = -min(exp(z)-1, 0) on ACT;
            # fused DVE scalar_tensor_tensor computes g = max(z, -un)
            un = hp.tile([128, 2 * CHUNK], F32, tag="u")
            nc.scalar.activation(un[:], t[:], AF.Relu, bias=1.0, scale=-1.0)
            nc.vector.scalar_tensor_tensor(
                out=g[:], in0=un[:], scalar=-1.0, in1=zfull,
                op0=ALU.mult, op1=ALU.max,
            )
        else:
            u = hp.tile([128, 2 * CHUNK], F32, tag="u")
            nc.vector.tensor_scalar(
                out=u[:], in0=t[:],
                scalar1=-1.0, scalar2=0.0, op0=ALU.add, op1=ALU.min,
            )
            nc.vector.tensor_tensor(out=g[:], in0=zfull, in1=u[:], op=ALU.max)

    def celu_add(zfull, r, q, idx=0):
        """celu(z) = r + q with r = relu(z), q = min(exp(z)-1, 0).
        The r+q add is folded into the consuming matmul (two rhs parts).
        r alternates ACT/DVE per chunk to balance engine load."""
        if celu_mode == "copy":
            nc.scalar.activation(r[:], zfull, AF.Copy)
            nc.vector.memset(q[:], 0.0)
            return
        t = hp.tile([128, 2 * CHUNK], F32, tag="t")
        nc.scalar.activation(t[:], zfull, AF.Exp)
        if idx % 2 == 0:
            nc.scalar.activation(r[:], zfull, AF.Relu)
        else:
            nc.vector.tensor_scalar(
                out=r[:], in0=zfull, scalar1=0.0, scalar2=None,
                op0=ALU.max, op1=ALU.bypass,
            )
        nc.vector.tensor_scalar(
            out=q[:], in0=t[:],
            scalar1=-1.0, scalar2=0.0, op0=ALU.add, op1=ALU.min,
        )

    only = os.environ.get("ONLY", "full")  # dma | mm1 | full  (bisection modes)
    ecap_off = [0]
    for e in range(E):
        ecap_off.append(ecap_off[-1] + caps[e])
    chunks = [(e, c) for e in range(E) for c in range(caps[e] // CHUNK)]
    n = len(chunks)
    S = {}  # software-pipeline state per chunk index
    weights = None  # tiles of the expert currently being loaded (stage A)
    first_mm3 = [True]

    for i in range(n + max(int(os.environ.get("SKEW_B", "1")), int(os.environ.get("SKEW_C", "2")))):
        # ---- stage A(i): weights + x DMA, mm1, celu1 ----
        if i < n:
            e, c = chunks[i]
            if c == 0:
                w1s = wp.tile([128, 3, H1], MMDT, tag="w1")
                nc.sync.dma_start(
                    out=w1s[:],
                    in_=W1[e].rearrange("(kt kp) m -> kp kt m", kp=128).bitcast(MMDT),
                )
                w2s = wp.tile([128, 2, H2], MMDT, tag="w2")
                nc.sync.dma_start(out=w2s[:, 0, :], in_=W2[e][0:128, :].bitcast(MMDT))
                nc.sync.dma_start(
                    out=w2s[0:64, 1, :], in_=W2[e][128:192, :].bitcast(MMDT)
                )
                w3s = wp.tile([128, 2], MMDT, tag="w3")
                nc.sync.dma_start(out=w3s[:, 0:1], in_=W3[e][0:128, :].bitcast(MMDT))
                nc.sync.dma_start(out=w3s[0:32, 1:2], in_=W3[e][128:160, :].bitcast(MMDT))
                q2full = os.environ.get("Q2FULL", "0") == "1"
                w3n = wp.tile([128, 2 if q2full else 1], MMDT, tag="w3n")
                nc.vector.tensor_scalar(
                    out=w3n[:], in0=(w3s[:] if q2full else w3s[:, 0:1]),
                    scalar1=-1.0, scalar2=None, op0=ALU.mult, op1=ALU.bypass,
                )
                if with_bias:
                    b1s = wp.tile([1, H1], MMDT, tag="b1")
                    nc.sync.dma_start(out=b1s[:], in_=B1[e : e + 1, :].bitcast(MMDT))
                    b2s = wp.tile([1, H2], MMDT, tag="b2")
                    nc.sync.dma_start(out=b2s[:], in_=B2[e : e + 1, :].bitcast(MMDT))
                else:
                    b1s = b2s = None
                weights = (w1s, w2s, w3s, b1s, b2s, w3n)

            w1s, w2s, w3s, b1s, b2s, w3n = weights
            off = ecap_off[e] + c * CHUNK
            xa = xp.tile([128, 3, CHUNK], MMDT, tag="xa")
            nc.sync.dma_start(
                out=xa[:], in_=xT3[:, :, off : off + CHUNK].bitcast(MMDT)
            )
            if only == "dma":
                continue

            z1 = zp.tile([128, 2 * CHUNK], F32, tag="z1")
            m_specs1 = [(0, 128, z1[:, 0:CHUNK]), (128, 64, z1[0:64, CHUNK:])]
            for m0, msz, zslice in m_specs1:
                for kt in range(3):
                    nc.tensor.matmul(
                        zslice,
                        lhsT=w1s[:, kt, m0 : m0 + msz],
                        rhs=xa[:, kt, :],
                        start=(kt == 0),
                        stop=(kt == 2 and not with_bias),
                    )
                if with_bias:
                    nc.tensor.matmul(
                        zslice, lhsT=b1s[:, m0 : m0 + msz], rhs=ones[:],
                        start=False, stop=True,
                    )
            if only == "mm1":
                continue
            if dma_add:
                t = hp.tile([128, 2 * CHUNK], F32, tag="t")
                nc.scalar.activation(t[:], z1[:], AF.Exp)
                u1 = hp.tile([128, 2 * CHUNK], MMDT, tag="u1")
                nc.vector.tensor_scalar(
                    out=u1[:], in0=t[:],
                    scalar1=-1.0, scalar2=0.0, op0=ALU.add, op1=ALU.min,
                )
                g1 = gp.tile([128, 2 * CHUNK], MMDT, tag="g1")
                nc.vector.tensor_scalar(
                    out=g1[:], in0=z1[:], scalar1=0.0, scalar2=None,
                    op0=ALU.max, op1=ALU.bypass,
                )
                nc.gpsimd.dma_start(out=g1[:], in_=u1[:], accum_op=ALU.add)
                S[i] = {"g1": g1, "w": weights}
            elif os.environ.get("CELU1", "max") == "add":
                r1 = gp.tile([128, 2 * CHUNK], MMDT, tag="g1")
                q1 = gp.tile([128, 2 * CHUNK], MMDT, tag="q1")
                celu_add(z1[:], r1, q1, idx=i + 1)
                S[i] = {"g1": r1, "q1": q1, "w": weights}
            else:
                g1 = gp.tile([128, 2 * CHUNK], MMDT, tag="g1")
                celu_max(z1[:], g1)
                S[i] = {"g1": g1, "w": weights}

        # ---- stage B: mm2, celu2 ----
        j = i - int(os.environ.get("SKEW_B", "1"))
        if 0 <= j < n and only == "full":
            st = S[j]
            w1s, w2s, w3s, b1s, b2s, w3n = st["w"]
            g1 = st["g1"]
            z2 = z2p.tile([128, 2 * CHUNK], F32, tag="z2")
            m_specs2 = [(0, 128, z2[:, 0:CHUNK]), (128, 32, z2[0:32, CHUNK:])]
            g1_parts = [g1] + ([st["q1"]] if "q1" in st else [])
            for m0, msz, zslice in m_specs2:
                first = True
                for gi, gpart in enumerate(g1_parts):
                    last_part = gi == len(g1_parts) - 1
                    nc.tensor.matmul(
                        zslice, lhsT=w2s[:, 0, m0 : m0 + msz],
                        rhs=gpart[0:128, 0:CHUNK], start=first, stop=False,
                    )
                    first = False
                    nc.tensor.matmul(
                        zslice, lhsT=w2s[0:64, 1, m0 : m0 + msz],
                        rhs=gpart[0:64, CHUNK : 2 * CHUNK],
                        start=False, stop=(last_part and not with_bias),
                    )
                if with_bias:
                    nc.tensor.matmul(
                        zslice, lhsT=b2s[:, m0 : m0 + msz], rhs=ones[:],
                        start=False, stop=True,
                    )
            r2 = gp.tile([128, 2 * CHUNK], MMDT, tag="r2")
            if dma_add:
                t2 = hp.tile([128, 2 * CHUNK], F32, tag="t")
                nc.scalar.activation(t2[:], z2[:], AF.Exp)
                if j % 2 == 0:
                    nc.scalar.activation(r2[:], z2[:], AF.Relu)
                else:
                    nc.vector.tensor_scalar(
                        out=r2[:], in0=z2[:], scalar1=0.0, scalar2=None,
                        op0=ALU.max, op1=ALU.bypass,
                    )
                q2 = hp.tile([128, 2 * CHUNK], MMDT, tag="q2d")
                nc.vector.tensor_scalar(
                    out=q2[:], in0=t2[:],
                    scalar1=-1.0, scalar2=0.0, op0=ALU.add, op1=ALU.min,
                )
                nc.gpsimd.dma_start(out=r2[:], in_=q2[:], accum_op=ALU.add)
                st["r2"], st["q2"] = r2, None
            else:
                q2 = gp.tile([128, 2 * CHUNK], MMDT, tag="q2")
                t2 = hp.tile([128, 2 * CHUNK], F32, tag="t")
                nc.scalar.activation(t2[:], z2[:], AF.Exp)
                # r2 = relu(z2), packed on DVE (steady per-chunk rhythm)
                nc.vector.tensor_scalar(
                    out=r2[:], in0=z2[:], scalar1=0.0, scalar2=None,
                    op0=ALU.max, op1=ALU.bypass,
                )
                if os.environ.get("Q2FULL", "0") == "1":
                    # q2 fully on ACT as -q; negation folds into both w3n cols
                    nc.scalar.activation(q2[:], t2[:], AF.Relu, bias=1.0, scale=-1.0)
                else:
                    # q2 m0 half on ACT as -q = relu(1 - exp(z)); negation folds
                    # into the w3n lhsT of the consuming matmul
                    nc.scalar.activation(
                        q2[:, 0:CHUNK], t2[:, 0:CHUNK], AF.Relu, bias=1.0, scale=-1.0
                    )
                    # q2 m1 half on DVE, plain sign
                    nc.vector.tensor_scalar(
                        out=q2[0:32, CHUNK:], in0=t2[0:32, CHUNK:],
                        scalar1=-1.0, scalar2=0.0, op0=ALU.add, op1=ALU.min,
                    )
                st["r2"], st["q2"] = r2, q2

        # ---- stage C: mm3 accumulate ----
        k = i - int(os.environ.get("SKEW_C", "2"))
        if 0 <= k < n and only == "full":
            st = S.pop(k)
            w3s = st["w"][2]
            w3n = st["w"][5]
            parts = [p for p in ("r2", "q2") if st.get(p) is not None]
            for part in parts:
                g2 = st[part]
                neg = part == "q2"
                q2full = os.environ.get("Q2FULL", "0") == "1"
                k0_lhsT = w3n[:, 0:1] if neg else w3s[:, 0:1]
                k1_lhsT = (
                    w3n[0:32, 1:2] if (neg and q2full) else w3s[0:32, 1:2]
                )
                nc.tensor.matmul(
                    acc[:], lhsT=k0_lhsT, rhs=g2[0:128, 0:CHUNK],
                    start=first_mm3[0], stop=False,
                )
                first_mm3[0] = False
                nc.tensor.matmul(
                    acc[:], lhsT=k1_lhsT, rhs=g2[0:32, CHUNK : 2 * CHUNK],
                    start=False, stop=(k == n - 1 and part == parts[-1]),
                )

    res = cp.tile([1, 1], F32, tag="res")
    if only == "full":
        nc.vector.tensor_reduce(
            out=res[:], in_=acc[:], axis=mybir.AxisListType.X, op=ALU.add
        )
    else:
        nc.vector.memset(res[:], 0.0)
    nc.sync.dma_start(out=out, in_=res[:])


_GRAPH_CACHE = {}


def _get_graph(with_bias: bool, caps):
    key = (with_bias, tuple(caps))
    if key not in _GRAPH_CACHE:
        _GRAPH_CACHE[key] = _build_graph(with_bias, caps)
    return _GRAPH_CACHE[key]


def _celu64(v):
    return np.where(v > 0, v, np.expm1(np.minimum(v, 0.0)))


def prepare_in_maps(aev_inputs, atom_types, W1, b1, W2, b2, W3, b3):
    """Host routing: build per-core input maps + metadata for corrections."""
    ndt = _np_mmdt()
    aev = np.asarray(aev_inputs, dtype=np.float32)
    types = np.asarray(atom_types).astype(np.int64)
    W1f = np.asarray(W1, dtype=np.float32)
    b1 = np.asarray(b1, dtype=np.float32)
    W2f = np.asarray(W2, dtype=np.float32)
    b2 = np.asarray(b2, dtype=np.float32)
    W3f = np.asarray(W3, dtype=np.float32)
    b3 = np.asarray(b3, dtype=np.float32)
    W1 = np.ascontiguousarray(W1f.astype(ndt))
    W2 = np.ascontiguousarray(W2f.astype(ndt))
    W3 = np.ascontiguousarray(W3f.astype(ndt))

    with_bias = bool(np.any(b1) or np.any(b2))

    # per-type atom lists, split evenly over cores
    order = np.argsort(types, kind="stable")
    sorted_types = types[order]
    bounds = np.searchsorted(sorted_types, np.arange(E + 1))
    type_lists = [order[bounds[e] : bounds[e + 1]] for e in range(E)]

    # per-(core, expert) slices and adaptive per-expert capacities.
    # When a core's batch barely overflows a 512 boundary (<= SHED_MAX atoms),
    # shed the overflow to an exact f64 host computation instead of paying a
    # whole extra device chunk for it.
    SHED_MAX = 64
    slices = [[None] * E for _ in range(N_CORES)]
    n_real = np.zeros((N_CORES, E), dtype=np.int64)
    shed = []  # atom indices computed on the host
    for e in range(E):
        lst = type_lists[e]
        counts = [((len(lst) * (c + 1)) // N_CORES) - ((len(lst) * c) // N_CORES)
                  for c in range(N_CORES)]
        mx = max(counts)
        rem = mx % CHUNK
        if 0 < rem <= SHED_MAX:
            cap_e = (mx // CHUNK) * CHUNK
        else:
            cap_e = -(-mx // CHUNK) * CHUNK
        for c in range(N_CORES):
            lo = (len(lst) * c) // N_CORES
            hi = (len(lst) * (c + 1)) // N_CORES
            take = min(hi - lo, cap_e)
            slices[c][e] = lst[lo : lo + take]
            shed.append(lst[lo + take : hi])
            n_real[c, e] = take
    shed = np.concatenate(shed) if shed else np.zeros(0, dtype=np.int64)
    caps = tuple(
        int(-(-int(n_real[:, e].max()) // CHUNK) * CHUNK) for e in range(E)
    )
    offs = [0]
    for e in range(E):
        offs.append(offs[-1] + caps[e])

    # exact f64 energies for shed atoms (tiny: <= SHED_MAX * E * N_CORES atoms)
    shed_energy = 0.0
    if len(shed):
        xs = aev[shed].astype(np.float64)
        ts_ = types[shed]
        for e in range(E):
            m = ts_ == e
            if not m.any():
                continue
            h = _celu64(xs[m] @ W1f[e].astype(np.float64) + b1[e].astype(np.float64))
            h = _celu64(h @ W2f[e].astype(np.float64) + b2[e].astype(np.float64))
            y = h @ W3f[e].astype(np.float64)[:, 0] + float(b3[e][0])
            shed_energy += float(y.sum())

    in_maps = []
    for c in range(N_CORES):
        xcT = np.zeros((IN_DIM, offs[-1]), dtype=ndt)
        for e in range(E):
            idx = slices[c][e]
            xcT[:, offs[e] : offs[e] + len(idx)] = aev[idx].T.astype(ndt)
        m = {"xT": xcT, "W1": W1, "W2": W2, "W3": W3}
        if with_bias:
            m["B1"] = np.ascontiguousarray(b1.astype(ndt))
            m["B2"] = np.ascontiguousarray(b2.astype(ndt))
        in_maps.append(m)
    return in_maps, n_real, with_bias, (b1, W2f, b2, W3f, b3, shed_energy), caps


def postprocess(results, n_real, wdata, caps):
    """Sum core partials + exact f64 corrections for pads, b3, shed atoms."""
    b1, W2, b2, W3, b3, shed_energy = wdata
    total0 = shed_energy
    total = total0
    for c in range(N_CORES):
        total += float(results[c]["out"][0, 0])
    counts_e = n_real.sum(axis=0)
    pads_e = np.array([N_CORES * caps[e] - counts_e[e] for e in range(E)])
    for e in range(E):
        h1 = _celu64(b1[e].astype(np.float64))
        z2 = h1 @ W2[e].astype(np.float64) + b2[e].astype(np.float64)
        y0dev = _celu64(z2) @ W3[e].astype(np.float64)[:, 0]
        total -= float(pads_e[e]) * float(y0dev)
        total += float(counts_e[e]) * float(b3[e][0])
    return np.asarray(total, dtype=np.float32)


def kernel(aev_inputs, atom_types, W1, b1, W2, b2, W3, b3):
    in_maps, n_real, with_bias, wdata, caps = prepare_in_maps(
        aev_inputs, atom_types, W1, b1, W2, b2, W3, b3
    )
    nc = _get_graph(with_bias, caps)
    results = bass_utils.run_bass_kernel_spmd(
        nc, in_maps, core_ids=list(range(N_CORES))
    ).results
    return postprocess(results, n_real, wdata, caps)



# revision 13
# speedup vs baseline: 2.3388x; 2.3388x over previous
"""ANI-style MoE routing kernel for 8 Trainium2 NeuronCores — v2.

Strategy (data-parallel + host routing):
  - Host: sort atoms by type, split each type's list evenly across 8 cores,
    build per-core per-expert contiguous batches padded to per-expert
    capacities (multiples of 1024).  Batches are feature-major bf16
    [384, sum(caps)].  Small overflows (<= SHED_MAX atoms) are computed
    exactly on the host in f64 instead of paying a whole device block.
  - Device (per core), per 1024-atom block, dim-major:
      mm1 (PE): z1 = W1^T x.  H1=192 -> m0 [128,1024] + m1 [64,1024]
        folded to [128,512] (two 512-atom column halves stacked on
        partitions 0/64) so pointwise ops touch zero garbage lanes.
      celu1: t1 = exp(z1) [ACT]; u1 = min(t1-1,0) [DVE ts 2x]; g1 =
        max(z1,u1) -> bf16 [DVE tt].
      mm2 (PE): z2 = W2^T g1.  H2=160 -> m0 [128,1024] + m1 [32,1024]
        folded to [64,512] (quarters at partition 0/32 x column halves).
      layer 3 is linear and the final output is a scalar, so NO mm3:
        per-dim sums of celu(z2) suffice.  celu(z2) = max(z2, u2) with
        u2 = min(exp(z2)-1, 0), so one DVE scalar_tensor_tensor /
        tensor_tensor_reduce pass with accum_out yields the per-block
        per-dim sums directly.  The u2 helper runs on ACT as
        relu(1-exp(z2)) (= -u2, sign folded into the STT) for m0 and on
        DVE for m1, balancing engine load.  Sum columns land in an SBUF
        tile, DMA'd out once at the end.
  - Host: S_e[dim] = sum of block columns; energy = sum_e w3[e]^T S_e in
    f64, + b3*counts + shed energies.

Zero-bias (always true for this problem's init) makes padding rows
self-cancelling: z=0 -> celu contribution exactly 0.  The general-bias
path adds per-layer bias matmuls and corrects pads on the host.
"""

import os
import sys

import numpy as np

try:
    import concourse.bass as bass  # noqa: F401
except ImportError:  # pragma: no cover
    sys.path.insert(0, "/opt/trn_rl_repo")
    import concourse.bass as bass  # noqa: F401

import concourse.mybir as mybir
import concourse.tile as tile
from concourse import bacc
from concourse import bass_utils

IN_DIM = 384
H1 = 192
H2 = 160
E = 4
N_CORES = 8
N_ATOMS = 262144

BLOCK = 1024
HB = 512  # half block

F32 = mybir.dt.float32
BF16 = mybir.dt.bfloat16
AF = mybir.ActivationFunctionType
ALU = mybir.AluOpType

# engine assignment knobs (A/B-testable)
UN2M0 = os.environ.get("UN2M0", "act")  # act | dve
U2M1 = os.environ.get("U2M1", "dve")  # dve | pool
U1_POOL = os.environ.get("U1_POOL", "0") == "1"


def _build_graph(with_bias: bool, caps, repeat: int = 1):
    nc = bacc.Bacc(
        "TRN2",
        target_bir_lowering=False,
        debug=False,
        enable_asserts=False,
        num_devices=N_CORES,
    )
    total_cap = sum(caps)
    nb = total_cap // BLOCK
    xT = nc.dram_tensor("xT", [IN_DIM, total_cap], BF16, kind="ExternalInput").ap()
    W1 = nc.dram_tensor("W1", [E, IN_DIM, H1], BF16, kind="ExternalInput").ap()
    W2 = nc.dram_tensor("W2", [E, H1, H2], BF16, kind="ExternalInput").ap()
    B1 = B2 = None
    if with_bias:
        B1 = nc.dram_tensor("B1", [E, H1], BF16, kind="ExternalInput").ap()
        B2 = nc.dram_tensor("B2", [E, H2], BF16, kind="ExternalInput").ap()
    outS = nc.dram_tensor("outS", [128, 2 * nb], F32, kind="ExternalOutput").ap()

    with tile.TileContext(nc) as tc:
        _emit(tc, xT, W1, W2, B1, B2, outS, with_bias, caps, repeat)
    nc.compile()
    return nc


def _emit(tc, xT, W1, W2, B1, B2, outS, with_bias, caps, repeat=1):
    import contextlib

    nc = tc.nc
    xT3 = xT.rearrange("(kt kp) n -> kp kt n", kp=128)  # [128, 3, total]
    nb = sum(caps) // BLOCK

    with (
        tc.tile_pool(name="wpool", bufs=1) as wp,
        tc.tile_pool(name="xpool", bufs=int(os.environ.get("X_BUFS", "3"))) as xp,
        tc.tile_pool(name="t1pool", bufs=2) as t1p,
        tc.tile_pool(name="u1pool", bufs=2) as u1p,
        tc.tile_pool(name="g1pool", bufs=2) as g1p,
        tc.tile_pool(name="t2pool", bufs=2) as t2p,
        tc.tile_pool(name="u2pool", bufs=2) as u2p,
        tc.tile_pool(name="cpool", bufs=1) as cp,
        tc.tile_pool(name="spool", bufs=1) as sp,
        tc.tile_pool(name="z1m0p", bufs=2, space="PSUM") as z1m0p,
        tc.tile_pool(name="z1m1p", bufs=1, space="PSUM") as z1m1p,
        tc.tile_pool(name="z2m0p", bufs=1, space="PSUM") as z2m0p,
        tc.tile_pool(name="z2m1p", bufs=1, space="PSUM") as z2m1p,
    ):
        # ---- persistent tiles (outside the timing repeat loop) ----
        w1s, w2s, b1s, b2s = [], [], [], []
        for e in range(E):
            w1 = wp.tile([128, 3, H1], BF16, tag=f"w1_{e}")
            nc.sync.dma_start(
                out=w1[:], in_=W1[e].rearrange("(kt kp) m -> kp kt m", kp=128)
            )
            w2 = wp.tile([128, 2, H2], BF16, tag=f"w2_{e}")
            nc.sync.dma_start(out=w2[:, 0, :], in_=W2[e][0:128, :])
            # kt1 weights duplicated at partition bases 0 and 64: matmul
            # requires lhsT.base_partition() == rhs.base_partition(), and
            # g1k1's two atom-halves live at partitions 0:64 / 64:128.
            nc.sync.dma_start(out=w2[0:64, 1, :], in_=W2[e][128:192, :])
            nc.sync.dma_start(out=w2[64:128, 1, :], in_=W2[e][128:192, :])
            w1s.append(w1)
            w2s.append(w2)
            if with_bias:
                b1 = wp.tile([1, H1], BF16, tag=f"b1_{e}")
                nc.sync.dma_start(out=b1[:], in_=B1[e : e + 1, :])
                b2 = wp.tile([1, H2], BF16, tag=f"b2_{e}")
                nc.sync.dma_start(out=b2[:], in_=B2[e : e + 1, :])
                b1s.append(b1)
                b2s.append(b2)
        ones = None
        if with_bias:
            ones = cp.tile([1, HB], BF16, tag="ones")
            nc.vector.memset(ones[:], 1.0)
        junkD = cp.tile([128, BLOCK], BF16, tag="junkD")
        SD = sp.tile([128, 2 * nb], F32, tag="SD")
        nc.vector.memset(SD[:], 0.0)

        loop_cm = tc.For_i(0, repeat, 1) if repeat > 1 else contextlib.nullcontext()
        with loop_cm:
            _emit_body(
                tc, xT3, w1s, w2s, b1s, b2s, ones, junkD, SD, with_bias, caps,
                xp, t1p, u1p, g1p, t2p, u2p, z1m0p, z1m1p, z2m0p, z2m1p,
            )
        nc.sync.dma_start(out=outS, in_=SD[:])


def _emit_body(
    tc, xT3, w1s, w2s, b1s, b2s, ones, junkD, SD, with_bias, caps,
    xp, t1p, u1p, g1p, t2p, u2p, z1m0p, z1m1p, z2m0p, z2m1p,
):
    nc = tc.nc
    ONLY = os.environ.get("ONLY", "full")
    nb = sum(caps) // BLOCK
    block_expert = []
    for e in range(E):
        block_expert += [e] * (caps[e] // BLOCK)

    PREFETCH = 2
    S = {}  # per-block pipeline state

    def dma_x(i):
        xa = xp.tile([128, 3, BLOCK], BF16, tag="xa")
        nc.sync.dma_start(
            out=xa[:], in_=xT3[:, :, i * BLOCK : (i + 1) * BLOCK]
        )
        return xa

    for i in range(min(PREFETCH, nb)):
        S[i] = {"xa": dma_x(i)}

    for i in range(nb + 2):
        # ---- stage P1(i-1): celu1 pointwise ----
        j = i - 1
        if 0 <= j < nb and ONLY in ("p1", "mm2", "full"):
            st = S[j]
            z1m0, z1m1 = st["z1m0"], st["z1m1"]
            t1 = t1p.tile([128, 1536], F32, tag="t1")
            # m1 first: unblocks mm1m1(i) (single-buffered Z1M1) earliest
            nc.scalar.activation(t1[:, 1024:1536], z1m1[:], AF.Exp)
            nc.scalar.activation(t1[:, 0:1024], z1m0[:], AF.Exp)
            u1 = u1p.tile([128, 1536], F32, tag="u1")
            g1k0 = g1p.tile([128, BLOCK], BF16, tag="g1k0")
            g1k1 = g1p.tile([128, HB], BF16, tag="g1k1")
            u1eng = nc.gpsimd if U1_POOL else nc.vector
            u1eng.tensor_scalar(
                out=u1[:, 1024:1536], in0=t1[:, 1024:1536],
                scalar1=-1.0, scalar2=0.0, op0=ALU.add, op1=ALU.min,
            )
            nc.vector.tensor_tensor(
                out=g1k1[:], in0=z1m1[:], in1=u1[:, 1024:1536], op=ALU.max
            )
            u1eng.tensor_scalar(
                out=u1[:, 0:1024], in0=t1[:, 0:1024],
                scalar1=-1.0, scalar2=0.0, op0=ALU.add, op1=ALU.min,
            )
            nc.vector.tensor_tensor(
                out=g1k0[:], in0=z1m0[:], in1=u1[:, 0:1024], op=ALU.max
            )
            st["g1k0"], st["g1k1"] = g1k0, g1k1

        # ---- stage P2(i-2): layer-2 pointwise + fused celu sums ----
        k = i - 2
        if 0 <= k < nb and ONLY == "full":
            st = S.pop(k)
            z2m0, z2m1 = st["z2m0"], st["z2m1"]
            t2 = t2p.tile([128, 1536], F32, tag="t2")
            u2 = u2p.tile([128, 1536], F32, tag="u2")
            nc.scalar.activation(t2[:, 0:1024], z2m0[:], AF.Exp)
            nc.scalar.activation(t2[0:64, 1024:1536], z2m1[:], AF.Exp)
            if UN2M0 == "act":
                # un2m0 = relu(1 - t2) = -u2 on ACT; sign folded into STT
                nc.scalar.activation(
                    u2[:, 0:1024], t2[:, 0:1024], AF.Relu, bias=1.0, scale=-1.0
                )
                nc.vector.scalar_tensor_tensor(
                    out=junkD[:], in0=u2[:, 0:1024], scalar=-1.0, in1=z2m0[:],
                    op0=ALU.mult, op1=ALU.max,
                    accum_out=SD[:, 2 * k : 2 * k + 1],
                )
            else:
                nc.vector.tensor_scalar(
                    out=u2[:, 0:1024], in0=t2[:, 0:1024],
                    scalar1=-1.0, scalar2=0.0, op0=ALU.add, op1=ALU.min,
                )
                nc.vector.scalar_tensor_tensor(
                    out=junkD[:], in0=u2[:, 0:1024], scalar=1.0, in1=z2m0[:],
                    op0=ALU.mult, op1=ALU.max,
                    accum_out=SD[:, 2 * k : 2 * k + 1],
                )
            u2m1eng = nc.gpsimd if U2M1 == "pool" else nc.vector
            u2m1eng.tensor_scalar(
                out=u2[0:64, 1024:1536], in0=t2[0:64, 1024:1536],
                scalar1=-1.0, scalar2=0.0, op0=ALU.add, op1=ALU.min,
            )
            nc.vector.scalar_tensor_tensor(
                out=junkD[0:64, 0:512], in0=u2[0:64, 1024:1536], scalar=1.0,
                in1=z2m1[:], op0=ALU.mult, op1=ALU.max,
                accum_out=SD[0:64, 2 * k + 1 : 2 * k + 2],
            )

        # ---- stage MM1(i) ----
        if i < nb and ONLY != "dma":
            st = S[i]
            e = block_expert[i]
            w1 = w1s[e]
            xa = st["xa"]
            z1m0 = z1m0p.tile([128, BLOCK], F32, tag="z1m0")
            z1m1 = z1m1p.tile([128, HB], F32, tag="z1m1")
            last = 2 if not with_bias else -1
            for kt in range(3):
                for h in range(2):
                    nc.tensor.matmul(
                        z1m0[:, HB * h : HB * (h + 1)],
                        lhsT=w1[:, kt, 0:128],
                        rhs=xa[:, kt, HB * h : HB * (h + 1)],
                        start=(kt == 0), stop=(kt == last),
                    )
                for h in range(2):
                    nc.tensor.matmul(
                        z1m1[64 * h : 64 * (h + 1), :],
                        lhsT=w1[:, kt, 128:192],
                        rhs=xa[:, kt, HB * h : HB * (h + 1)],
                        start=(kt == 0), stop=(kt == last),
                    )
            if with_bias:
                b1 = b1s[e]
                for h in range(2):
                    nc.tensor.matmul(
                        z1m0[:, HB * h : HB * (h + 1)], lhsT=b1[:, 0:128],
                        rhs=ones[:], start=False, stop=True,
                    )
                for h in range(2):
                    nc.tensor.matmul(
                        z1m1[64 * h : 64 * (h + 1), :], lhsT=b1[:, 128:192],
                        rhs=ones[:], start=False, stop=True,
                    )
            st["z1m0"], st["z1m1"] = z1m0, z1m1

        # ---- stage MM2(i-1) ----
        if 0 <= j < nb and ONLY in ("mm2", "full"):
            st = S[j]
            e = block_expert[j]
            w2 = w2s[e]
            g1k0, g1k1 = st["g1k0"], st["g1k1"]
            z2m0 = z2m0p.tile([128, BLOCK], F32, tag="z2m0")
            z2m1 = z2m1p.tile([64, HB], F32, tag="z2m1")
            last_stop = not with_bias
            MM2P = int(os.environ.get("MM2P", "15"))
            _solo = MM2P != 15
            # kt0 m0 (LDW w2[:,0,0:128])
            for h in (range(2) if MM2P & 1 else []):
                nc.tensor.matmul(
                    z2m0[:, HB * h : HB * (h + 1)], lhsT=w2[:, 0, 0:128],
                    rhs=g1k0[:, HB * h : HB * (h + 1)], start=True, stop=(False or _solo),
                )
            # kt0 m1 (LDW w2[:,0,128:160]): atom-half h -> parts 32h,
            # cols 0:512 (k-row base 0; positions (0,0)/(0,32))
            for h in (range(2) if MM2P & 2 else []):
                nc.tensor.matmul(
                    z2m1[32 * h : 32 * h + 32, 0:HB],
                    lhsT=w2[:, 0, 128:160],
                    rhs=g1k0[:, HB * h : HB * (h + 1)],
                    start=True, stop=(False or _solo),
                )
            # kt1 m0; g1k1 parts: h half of atoms, lhsT at matching base
            for h in (range(2) if MM2P & 4 else []):
                nc.tensor.matmul(
                    z2m0[:, HB * h : HB * (h + 1)],
                    lhsT=w2[64 * h : 64 * h + 64, 1, 0:128],
                    rhs=g1k1[64 * h : 64 * (h + 1), :],
                    start=(False or _solo), stop=last_stop,
                )
            # kt1 m1: positions (0,0) h0 / (64,32) h1 — the safe diagonal
            for h in (range(2) if MM2P & 8 else []):
                nc.tensor.matmul(
                    z2m1[32 * h : 32 * h + 32, 0:HB],
                    lhsT=w2[64 * h : 64 * h + 64, 1, 128:160],
                    rhs=g1k1[64 * h : 64 * (h + 1), 0:HB],
                    start=(False or _solo), stop=last_stop,
                )
            if with_bias:
                b2 = b2s[e]
                for h in range(2):
                    nc.tensor.matmul(
                        z2m0[:, HB * h : HB * (h + 1)], lhsT=b2[:, 0:128],
                        rhs=ones[:], start=False, stop=True,
                    )
                for h in range(2):
                    nc.tensor.matmul(
                        z2m1[32 * h : 32 * h + 32, 0:HB],
                        lhsT=b2[:, 128:160],
                        rhs=ones[:], start=False, stop=True,
                    )
            st["z2m0"], st["z2m1"] = z2m0, z2m1

        # ---- prefetch ----
        nxt = i + PREFETCH
        if nxt < nb:
            S[nxt] = {"xa": dma_x(nxt)}


_GRAPH_CACHE = {}


def _get_graph(with_bias: bool, caps):
    key = (with_bias, tuple(caps))
    if key not in _GRAPH_CACHE:
        _GRAPH_CACHE[key] = _build_graph(with_bias, caps)
    return _GRAPH_CACHE[key]


def _celu64(v):
    return np.where(v > 0, v, np.expm1(np.minimum(v, 0.0)))


def prepare_in_maps(aev_inputs, atom_types, W1, b1, W2, b2, W3, b3):
    """Host routing: build per-core input maps + metadata for decode."""
    import ml_dtypes

    ndt = ml_dtypes.bfloat16
    aev = np.asarray(aev_inputs, dtype=np.float32)
    types = np.asarray(atom_types).astype(np.int64)
    W1f = np.asarray(W1, dtype=np.float32)
    b1 = np.asarray(b1, dtype=np.float32)
    W2f = np.asarray(W2, dtype=np.float32)
    b2 = np.asarray(b2, dtype=np.float32)
    W3f = np.asarray(W3, dtype=np.float32)
    b3 = np.asarray(b3, dtype=np.float32)
    W1b = np.ascontiguousarray(W1f.astype(ndt))
    W2b = np.ascontiguousarray(W2f.astype(ndt))

    with_bias = bool(np.any(b1) or np.any(b2))

    order = np.argsort(types, kind="stable")
    sorted_types = types[order]
    bounds = np.searchsorted(sorted_types, np.arange(E + 1))
    type_lists = [order[bounds[e] : bounds[e + 1]] for e in range(E)]

    SHED_MAX = 192
    slices = [[None] * E for _ in range(N_CORES)]
    n_real = np.zeros((N_CORES, E), dtype=np.int64)
    shed = []
    caps = []
    for e in range(E):
        lst = type_lists[e]
        counts = [
            ((len(lst) * (c + 1)) // N_CORES) - ((len(lst) * c) // N_CORES)
            for c in range(N_CORES)
        ]
        mx = max(counts)
        rem = mx % BLOCK
        if 0 < rem <= SHED_MAX:
            cap_e = (mx // BLOCK) * BLOCK
        else:
            cap_e = -(-mx // BLOCK) * BLOCK
        caps.append(cap_e)
        for c in range(N_CORES):
            lo = (len(lst) * c) // N_CORES
            hi = (len(lst) * (c + 1)) // N_CORES
            take = min(hi - lo, cap_e)
            slices[c][e] = lst[lo : lo + take]
            shed.append(lst[lo + take : hi])
            n_real[c, e] = take
    shed = np.concatenate(shed) if shed else np.zeros(0, dtype=np.int64)
    caps = tuple(caps)
    offs = np.cumsum([0] + list(caps))

    shed_energy = 0.0
    if len(shed):
        xs = aev[shed].astype(np.float64)
        ts_ = types[shed]
        for e in range(E):
            m = ts_ == e
            if not m.any():
                continue
            h = _celu64(xs[m] @ W1f[e].astype(np.float64) + b1[e].astype(np.float64))
            h = _celu64(h @ W2f[e].astype(np.float64) + b2[e].astype(np.float64))
            y = h @ W3f[e].astype(np.float64)[:, 0] + float(b3[e][0])
            shed_energy += float(y.sum())

    in_maps = []
    for c in range(N_CORES):
        xcT = np.zeros((IN_DIM, int(offs[-1])), dtype=ndt)
        for e in range(E):
            idx = slices[c][e]
            xcT[:, int(offs[e]) : int(offs[e]) + len(idx)] = aev[idx].T.astype(ndt)
        m = {"xT": xcT, "W1": W1b, "W2": W2b}
        if with_bias:
            m["B1"] = np.ascontiguousarray(b1.astype(ndt))
            m["B2"] = np.ascontiguousarray(b2.astype(ndt))
        in_maps.append(m)
    return in_maps, n_real, with_bias, (b1, W2f, b2, W3f, b3, shed_energy), caps


def postprocess(results, n_real, wdata, caps):
    """Decode per-block sum columns -> per-expert per-dim sums -> energy."""
    b1, W2f, b2, W3f, b3, shed_energy = wdata
    nb = sum(caps) // BLOCK
    block_expert = []
    for e in range(E):
        block_expert += [e] * (caps[e] // BLOCK)

    S = np.zeros((E, H2), dtype=np.float64)  # sum of celu(z2) per expert/dim
    for c in range(N_CORES):
        D = np.asarray(results[c]["outS"], dtype=np.float64)  # [128, 2nb]
        for b in range(nb):
            e = block_expert[b]
            S[e, 0:128] += D[:, 2 * b]
            # m1 col: partitions 0:64, dim = 128 + p % 32
            S[e, 128:160] += D[0:64, 2 * b + 1].reshape(2, 32).sum(axis=0)

    total = shed_energy
    counts_e = n_real.sum(axis=0)
    pads_e = np.array([N_CORES * caps[e] - counts_e[e] for e in range(E)])
    for e in range(E):
        w3 = W3f[e].astype(np.float64)[:, 0]
        total += float(w3 @ S[e])
        total += float(counts_e[e]) * float(b3[e][0])
        if pads_e[e]:
            # device pads contribute celu(z2_0) per dim; subtract (f64 model)
            h1 = _celu64(b1[e].astype(np.float64))
            z2_0 = h1 @ W2f[e].astype(np.float64) + b2[e].astype(np.float64)
            total -= float(pads_e[e]) * float(w3 @ _celu64(z2_0))
    return np.asarray(total, dtype=np.float32)


def kernel(aev_inputs, atom_types, W1, b1, W2, b2, W3, b3):
    in_maps, n_real, with_bias, wdata, caps = prepare_in_maps(
        aev_inputs, atom_types, W1, b1, W2, b2, W3, b3
    )
    nc = _get_graph(with_bias, caps)
    results = bass_utils.run_bass_kernel_spmd(
        nc, in_maps, core_ids=list(range(N_CORES))
    ).results
    return postprocess(results, n_real, wdata, caps)
